# revision 3
# baseline (speedup 1.0000x reference)
"""Trainium2 Bass kernel for nn_ModelNew_17411797418162.

Computation (per (b,s) sample):
  mixed = h_res @ x            # [4,4] @ [4,1024]
  out   = mixed * h_out[None,:] + h_post[:,None] * x

Sharding: pure data parallel over the leading batch dim B=8 -> 1 batch/core.

Per-core design (memory-bound). All device I/O is fp16 (host-side cast;
rel err ~1.5e-3 vs fp32 reference, well under the 2e-2 gate), which halves
HBM traffic vs the fp32 baseline: x 16MB + out 16MB + h_out 4MB +
weights 4MB ~= 40MB -> ~135-145us DMA roofline/core.

- Flatten (s, stream) -> rows: x/out [8192, 1024] fp16; loads and stores
  are contiguous 2KB-per-partition-line DMAs.
- Rewrite out = (h_res @ (x * h_out_bcast)) + diag(h_post) @ x:
  * h_out is broadcast onto the 4 stream rows by a constant 0/1 matmul on
    the PE (e4), landing in PSUM.
  * DVE does the single elementwise op: y = x * h_out_bcast (fp16 out).
  * The per-sample 4x4 GEMM runs as exact block-diagonal [128,128] fp16
    matmuls covering 32 samples each, and the h_post residual term is a
    *diagonal* [128,128] fp16 matmul accumulated into the same PSUM group
    (start/stop flags), so no separate vector add is needed.
  * ACT evacuates the final PSUM to fp16 SBUF; one store DMA per 512 rows.
- Per-[128,1024] block engine cost: DVE ~1.2us, ACT ~0.95us, PE ~1.3us
  (6 fp16 matmuls), all under the ~2.2us DMA slot -> DMA-bound.
"""
import numpy as np

import concourse.bacc as bacc
import concourse.tile as tile
import concourse.mybir as mybir
from concourse.bass_utils import run_bass_kernel_spmd

B, S, N, D = 8, 2048, 4, 1024
NCORES = 8
ROWS = S * N              # 8192 flattened rows per core
NSB = 16                  # super-blocks per core (512 rows each)
SUBS = 4                  # sub-blocks (32 samples x 4 streams) per super-block
F32 = mybir.dt.float32
FP16 = mybir.dt.float16

_cache = {}


def build_program(iters: int = 1, mode: str = "full"):
    """Build the SPMD Bass program (one core's view). Cached per (iters, mode).

    mode: "full" = real kernel; ablations for bottleneck isolation:
      "dma"   = loads + store only (wrong output values)
      "nodve" = full minus the DVE multiply (wrong values)
      "nope"  = loads, DVE mult on x, ACT evac of x, store (no matmuls)
    """
    if (iters, mode) in _cache:
        return _cache[(iters, mode)]

    nc = bacc.Bacc("TRN2", target_bir_lowering=False, debug=False)
    x = nc.dram_tensor("x", [ROWS, D], FP16, kind="ExternalInput")
    w = nc.dram_tensor("w", [128, 64 * 128], FP16, kind="ExternalInput")
    dg = nc.dram_tensor("dg", [128, 64 * 128], FP16, kind="ExternalInput")
    ho = nc.dram_tensor("ho", [S, D], FP16, kind="ExternalInput")
    e4 = nc.dram_tensor("e4", [128, 128], FP16, kind="ExternalInput")
    out = nc.dram_tensor("out", [ROWS, D], FP16, kind="ExternalOutput")

    with tile.TileContext(nc) as tc:
        with (
            tc.tile_pool(name="const", bufs=1) as cpool,
            tc.tile_pool(name="big", bufs=3) as bpool,
            tc.tile_pool(name="mid", bufs=4) as mpool,
            tc.tile_pool(name="psum", bufs=4, space="PSUM") as ppool,
        ):
            e4_t = cpool.tile([128, 128], FP16)
            nc.gpsimd.dma_start(e4_t[:], e4.ap())
            # resident block-diagonal mixing weights + diagonal h_post weights
            w_all = cpool.tile([128, 64 * 128], FP16)
            nc.gpsimd.dma_start(w_all[:], w.ap())
            d_all = cpool.tile([128, 64 * 128], FP16)
            nc.gpsimd.dma_start(d_all[:], dg.ap())

            def body():
                for sb in range(NSB):
                    # x rows 512*sb .. 512*(sb+1), tiled [p=128, (k=4, d=1024)]
                    x_t = bpool.tile([128, SUBS * D], FP16, tag="x")
                    src = x.ap()[512 * sb:512 * (sb + 1), :].rearrange(
                        "(k p) d -> p k d", k=SUBS)
                    nc.sync.dma_start(
                        x_t[:].rearrange("p (k d) -> p k d", k=SUBS), src)

                    # h_out fp16 for these 128 samples (partition = sample)
                    ho_t = mpool.tile([128, D], FP16, tag="ho")
                    nc.gpsimd.dma_start(ho_t[:], ho.ap()[128 * sb:128 * (sb + 1)])

                    out_sb = bpool.tile([128, SUBS * D], FP16, tag="out_sb")

                    for k in range(SUBS):
                        if mode == "dma":
                            continue
                        blk = SUBS * sb + k
                        xk = x_t[:, D * k:D * (k + 1)]

                        if mode in ("full", "nodve"):
                            # broadcast h_out onto stream rows via 0/1 matmul
                            psh = ppool.tile([128, D], F32, tag="ps")
                            for c in range(2):
                                nc.tensor.matmul(
                                    psh[:, 512 * c:512 * (c + 1)],
                                    e4_t[32 * k:32 * (k + 1), :],
                                    ho_t[32 * k:32 * (k + 1),
                                         512 * c:512 * (c + 1)],
                                    start=True, stop=True,
                                    tile_position=(32 * k, 0))

                        y_t = mpool.tile([128, D], FP16, tag="y")
                        if mode == "full":
                            nc.vector.tensor_mul(y_t[:], xk, psh)
                        elif mode == "nope":
                            nc.vector.tensor_mul(y_t[:], xk, xk)

                        if mode in ("full", "nodve"):
                            # "nodve": feed raw x to the mix matmul instead of y
                            rhs = y_t[:] if mode == "full" else xk
                            ps = ppool.tile([128, D], F32, tag="ps")
                            lhsW = w_all[:, 128 * blk:128 * (blk + 1)]
                            lhsD = d_all[:, 128 * blk:128 * (blk + 1)]
                            for c in range(2):
                                nc.tensor.matmul(
                                    ps[:, 512 * c:512 * (c + 1)],
                                    lhsW,
                                    rhs[:, 512 * c:512 * (c + 1)],
                                    start=True, stop=False)
                                nc.tensor.matmul(
                                    ps[:, 512 * c:512 * (c + 1)],
                                    lhsD,
                                    x_t[:, D * k + 512 * c:
                                        D * k + 512 * (c + 1)],
                                    start=False, stop=True)
                            nc.scalar.copy(out_sb[:, D * k:D * (k + 1)], ps[:])
                        else:
                            nc.scalar.copy(out_sb[:, D * k:D * (k + 1)], y_t[:])

                    src_sb = x_t if mode == "dma" else out_sb
                    dst = out.ap()[512 * sb:512 * (sb + 1), :].rearrange(
                        "(k p) d -> p k d", k=SUBS)
                    nc.scalar.dma_start(
                        dst, src_sb[:].rearrange("p (k d) -> p k d", k=SUBS))

            if iters == 1:
                body()
            else:
                with tc.For_i(0, iters, 1):
                    body()

    nc.compile()
    _cache[(iters, mode)] = nc
    return nc


def _f16(a):
    """fp16 cast with subnormal flush (HW engines may flush; make it exact)."""
    h = np.asarray(a, np.float32).astype(np.float16)
    h[np.abs(h.astype(np.float32)) < 2.0 ** -14] = 0
    return h


def make_in_maps(x, h_res, h_out, h_post):
    """Split full inputs into per-core input maps (host-side, layout only)."""
    x = np.ascontiguousarray(x, dtype=np.float32)
    h_res = np.ascontiguousarray(h_res, dtype=np.float32)
    h_out = np.ascontiguousarray(h_out, dtype=np.float32)
    h_post = np.ascontiguousarray(h_post, dtype=np.float32)

    # stream-replication matrix: e4[q, 4*(q%32)+i] = 1  (lhsT of the bcast
    # matmul; K-rows live at partitions 32k..32k+32 via tile_position)
    e4 = np.zeros((128, 128), np.float16)
    q = np.arange(128)
    for i in range(4):
        e4[q, 4 * (q % 32) + i] = 1.0

    p32 = np.arange(32)
    r128 = np.arange(128)

    in_maps = []
    for c in range(NCORES):
        xc = _f16(x[c].reshape(ROWS, D))
        # Block-diagonal mixing weights, laid out [r, (b, col)] so the DMA is
        # contiguous 16KB per partition: W[4p+j, b, 4p+i] = h_res[c, 32b+p, i, j]
        hr = _f16(h_res[c].reshape(64, 32, 4, 4))       # [b, p, i, j]
        W = np.zeros((128, 64, 128), np.float16)
        for i in range(4):
            for j in range(4):
                W[4 * p32 + j, :, 4 * p32 + i] = hr[:, p32, i, j].T
        # Diagonal h_post weights: Dg[r, b, r] = h_post[c, 32b + r//4, r%4]
        hpc = _f16(h_post[c].reshape(64, 32, 4))        # [b, p, i]
        Dg = np.zeros((128, 64, 128), np.float16)
        Dg[r128, :, r128] = hpc[:, r128 // 4, r128 % 4].T
        m = {
            "x": xc,
            "w": np.ascontiguousarray(W.reshape(128, 64 * 128)),
            "dg": np.ascontiguousarray(Dg.reshape(128, 64 * 128)),
            "ho": _f16(h_out[c]),
            "e4": e4,
        }
        in_maps.append(m)
    return in_maps


def kernel(x, h_res, h_out, h_post):
    nc = build_program(iters=1)
    in_maps = make_in_maps(x, h_res, h_out, h_post)
    res = run_bass_kernel_spmd(nc, in_maps, list(range(NCORES)))
    out = np.stack([res.results[c]["out"].reshape(S, N, D)
                    for c in range(NCORES)])
    return out.astype(np.float32)
